# revision 1
# baseline (speedup 1.0000x reference)
"""DeepWalk random-walk kernel for 8 Trainium2 NeuronCores.

Problem (hardcoded from spec): CSR graph with N=100000 nodes, fixed
out-degree 16 (indptr = arange(N+1)*16), indices[1.6M] int32 random,
rand_vals [10, 100000, 80] f32. Output walks [10, 100000, 80] f32 where
walks[w,n,t] = node id at step t (walks never die: deg==16 for all nodes).

Recurrence per walk: v_{t+1} = indices[v_t*16 + floor(u_t*16)], record v_t.

Strategy: shard the 100000 start nodes across 8 cores (12500 each ->
125000 walks/core). Walks live in SBUF as [128, 977] f32 slots
(walk j -> partition j%128, column j//128). Per step:
  - DVE computes q = v*16 + floor(u*16) exactly in f32 (floor built from
    round-to-nearest int cast + is_gt correction), casts to int32.
  - Pool engine issues 977 per-column indirect DMAs (SWDGE vector-indirect
    gather): out[p,k] = table_f32[q[p,k]] -- the table is the neighbor
    array pre-converted to f32 in DRAM (values < 2^24 so exact).
  - Sync engine streams u in (double-buffered) and records v out to DRAM.
80 steps via a hardware Fori loop (2-step unrolled body, ping-pong bufs).
Gathers, DVE index math, and the record DMA are all split into two
semaphore halves per step so half B's SDMA drain overlaps the next step's
half-A work; the remaining ~2.9 ms/step is the SDMA random-read latency
floor (128 4B DRAM reads/instr over 16 engines at ~360 ns each).
Host pre/post: pure layout reshapes + int->f32 table conversion.
"""

import sys

sys.path.insert(0, "/opt/trn_rl_repo")

import numpy as np

import concourse.bacc as bacc
import concourse.bass as bass
import concourse.mybir as mybir
from concourse import bass_utils
from concourse.bass import ds

N_NODES = 100000
DEGREE = 16
WALKS_PER_VERTEX = 10
WALK_LENGTH = 80
NCORES = 8
NSH = N_NODES // NCORES          # nodes per core
WALKS = WALKS_PER_VERTEX * NSH   # walks per core
P = 128
COLS = (WALKS + P - 1) // P      # 977
PAD = P * COLS                   # 125056
COLS_A = COLS // 2               # first-half columns (488)
COLS_B = COLS - COLS_A           # second half (489)
GINC_A = COLS_A * 16
GINC_B = COLS_B * 16

_cache = {}


def _build(n_steps):
    f32 = mybir.dt.float32
    i32 = mybir.dt.int32
    nc = bacc.Bacc("TRN2", debug=False)

    tab_d = nc.dram_tensor("tab", [N_NODES * DEGREE, 1], f32, kind="ExternalInput")
    u_d = nc.dram_tensor("u", [(n_steps + 2) * P, COLS], f32, kind="ExternalInput")
    v0_d = nc.dram_tensor("v0", [P, COLS], f32, kind="ExternalInput")
    w_d = nc.dram_tensor("walks", [n_steps * P, COLS], f32, kind="ExternalOutput")

    v_bufs = [nc.alloc_sbuf_tensor(f"vb{s}", [P, COLS], f32).ap() for s in (0, 1)]
    u_bufs = [nc.alloc_sbuf_tensor(f"ub{s}", [P, COLS], f32).ap() for s in (0, 1)]
    t1 = nc.alloc_sbuf_tensor("t1", [P, COLS], f32).ap()
    fl = nc.alloc_sbuf_tensor("fl", [P, COLS], f32).ap()
    gt = nc.alloc_sbuf_tensor("gt", [P, COLS], f32).ap()
    qf = nc.alloc_sbuf_tensor("qf", [P, COLS], f32).ap()
    ri = nc.alloc_sbuf_tensor("ri", [P, COLS], i32).ap()
    qi = nc.alloc_sbuf_tensor("qi", [P, COLS], i32).ap()

    uin_sem = nc.alloc_semaphore()
    dveA_sem = nc.alloc_semaphore()
    dveB_sem = nc.alloc_semaphore()
    gA_sem = nc.alloc_semaphore()
    gB_sem = nc.alloc_semaphore()
    outA_sem = nc.alloc_semaphore()
    outB_sem = nc.alloc_semaphore()

    # prologue: load v0 and u_0
    nc.sync.dma_start(v_bufs[0][:], v0_d.ap()[:, :]).then_inc(uin_sem, 16)
    nc.sync.dma_start(u_bufs[0][:], u_d.ap()[0:P, :]).then_inc(uin_sem, 16)
    nc.sync.wait_ge(uin_sem, 32)
    nc.vector.wait_ge(uin_sem, 32)

    ALL = [mybir.EngineType.Pool, mybir.EngineType.DVE, mybir.EngineType.SP]

    def step_body(i, s):
        # t = 2*i + s (ScalarValue expression); constants folded per s
        cur = v_bufs[s]
        nxt = v_bufs[1 - s]
        ucur = u_bufs[s]
        unxt = u_bufs[1 - s]

        t_gA = i * (2 * GINC_A) + s * GINC_A     # 16*COLS_A*t
        t_gB = i * (2 * GINC_B) + s * GINC_B
        t_1 = i * 2 + s                          # t
        t_16 = i * 32 + s * 16                   # 16*t
        row0 = i * (2 * P) + s * P               # 128*t

        # --- sync engine: record v_t (split in halves so gather-gen of the
        # next step only depends on its own half's record), prefetch u ---
        nc.sync.wait_ge(gA_sem, t_gA)
        nc.sync.dma_start(w_d.ap()[ds(row0, P), 0:COLS_A],
                          cur[:, 0:COLS_A]).then_inc(outA_sem, 16)
        nc.sync.wait_ge(gB_sem, t_gB)
        nc.sync.dma_start(w_d.ap()[ds(row0, P), COLS_A:COLS],
                          cur[:, COLS_A:COLS]).then_inc(outB_sem, 16)
        nc.sync.wait_ge(dveB_sem, t_1)
        nc.sync.dma_start(unxt[:], u_d.ap()[ds(row0 + P, P), :]).then_inc(uin_sem, 16)

        # --- DVE: q = v*16 + floor(u*16), in halves so gathers can start
        # on half A while half B of the previous step still drains ---
        nc.vector.wait_ge(uin_sem, i * 32 + s * 16 + 32)   # u_t present
        def half(sl, done_sem, done_val, inc_sem):
            nc.vector.wait_ge(done_sem, done_val)          # v_t half present; q half free
            nc.vector.tensor_scalar_mul(t1[:, sl], ucur[:, sl], 16.0)
            nc.vector.tensor_copy(ri[:, sl], t1[:, sl])    # round-to-nearest
            nc.vector.tensor_copy(fl[:, sl], ri[:, sl])    # back to f32 (exact)
            nc.vector.tensor_tensor(gt[:, sl], fl[:, sl], t1[:, sl], op=mybir.AluOpType.is_gt)
            nc.vector.tensor_tensor(fl[:, sl], fl[:, sl], gt[:, sl], op=mybir.AluOpType.subtract)
            nc.vector.tensor_scalar_mul(qf[:, sl], cur[:, sl], 16.0)
            nc.vector.tensor_tensor(qf[:, sl], qf[:, sl], fl[:, sl], op=mybir.AluOpType.add)
            nc.vector.tensor_copy(qi[:, sl], qf[:, sl]).then_inc(inc_sem, 1)
        half(slice(0, COLS_A), gA_sem, t_gA, dveA_sem)
        half(slice(COLS_A, COLS), gB_sem, t_gB, dveB_sem)

        # --- Pool: per-column indirect gathers, half A then half B ---
        nc.gpsimd.wait_ge(outA_sem, t_16)
        nc.gpsimd.wait_ge(dveA_sem, t_1 + 1)
        for k in range(COLS_A):
            nc.gpsimd.indirect_dma_start(
                out=nxt[:, k:k + 1],
                out_offset=None,
                in_=tab_d.ap()[:, :],
                in_offset=bass.IndirectOffsetOnAxis(ap=qi[:, k:k + 1], axis=0),
            ).then_inc(gA_sem, 16)
        nc.gpsimd.wait_ge(outB_sem, t_16)
        nc.gpsimd.wait_ge(dveB_sem, t_1 + 1)
        for k in range(COLS_A, COLS):
            nc.gpsimd.indirect_dma_start(
                out=nxt[:, k:k + 1],
                out_offset=None,
                in_=tab_d.ap()[:, :],
                in_offset=bass.IndirectOffsetOnAxis(ap=qi[:, k:k + 1], axis=0),
            ).then_inc(gB_sem, 16)

    assert n_steps % 2 == 0
    with nc.Fori(0, n_steps // 2, engines=ALL) as i:
        step_body(i, 0)
        step_body(i, 1)

    nc.sync.wait_ge(outA_sem, 16 * n_steps)
    nc.sync.wait_ge(outB_sem, 16 * n_steps)
    nc.sync.wait_ge(gA_sem, GINC_A * n_steps)
    nc.sync.wait_ge(gB_sem, GINC_B * n_steps)
    nc.all_engine_barrier()
    nc.finalize()
    return nc


def _get_nc(n_steps):
    if n_steps not in _cache:
        _cache[n_steps] = _build(n_steps)
    return _cache[n_steps]


def kernel(indptr, indices, rand_vals):
    indptr = np.asarray(indptr)
    indices = np.asarray(indices)
    rand_vals = np.asarray(rand_vals)
    W, N, L = rand_vals.shape
    assert (W, N) == (WALKS_PER_VERTEX, N_NODES) and L % 2 == 0
    # the kernel exploits the fixed out-degree structure
    assert np.array_equal(indptr, (np.arange(N + 1) * DEGREE).astype(np.int32))

    tab = np.ascontiguousarray(indices.astype(np.float32).reshape(-1, 1))

    in_maps = []
    for c in range(NCORES):
        sl = rand_vals[:, c * NSH:(c + 1) * NSH, :]           # [W, NSH, L]
        U = sl.reshape(WALKS, L)                               # walk-major j = w*NSH+n
        U_pad = np.zeros((PAD, L), np.float32)
        U_pad[:WALKS] = U
        # u_pre[t, p, f] = U_pad[f*128 + p, t]
        u_pre = U_pad.T.reshape(L, COLS, P).swapaxes(1, 2)     # [L, P, COLS]
        u_full = np.zeros(((L + 2) * P, COLS), np.float32)
        u_full[:L * P] = u_pre.reshape(L * P, COLS)

        j = np.arange(PAD)
        v0 = np.where(j < WALKS, c * NSH + (j % NSH), 0).astype(np.float32)
        v0 = v0.reshape(COLS, P).T.copy()                      # [P, COLS]

        in_maps.append({"tab": tab, "u": np.ascontiguousarray(u_full), "v0": v0})

    nc = _get_nc(L)
    res = bass_utils.run_bass_kernel_spmd(nc, in_maps, core_ids=list(range(NCORES)))

    out = np.empty((W, N, L), np.float32)
    for c in range(NCORES):
        w_t = res.results[c]["walks"]                          # [L*P, COLS]
        Wc = w_t.reshape(L, P, COLS).swapaxes(1, 2).reshape(L, PAD)[:, :WALKS]
        out[:, c * NSH:(c + 1) * NSH, :] = Wc.T.reshape(W, NSH, L)
    return out



# revision 2
# speedup vs baseline: 13.5997x; 13.5997x over previous
"""DeepWalk random-walk kernel for 8 Trainium2 NeuronCores — v2.

Pipelined single-step gathers. Per core: 125000 walks as [128, 977] SBUF
slots. Per step t: DVE computes q = 16*v + off (f32, exact: q < 1.6M < 2^24),
casts to i32; Pool issues 977 per-column indirect DMAs (128 descriptors
each, ~1.4us/instr = SWDGE floor); sync engine records v_t to DRAM and
streams off (uint8, host-precomputed floor(16*u)).

The step is split into NCH=8 column chunks pipelined so the Pool engine
never stalls: DVE computes chunk c addresses while Pool issues chunk c-1
gathers of the same step.
"""

import sys

sys.path.insert(0, "/opt/trn_rl_repo")

import numpy as np

import concourse.bacc as bacc
import concourse.bass as bass
import concourse.mybir as mybir
from concourse import bass_utils
from concourse.bass import ds

N_NODES = 100000
DEGREE = 16
WALKS_PER_VERTEX = 10
WALK_LENGTH = 80
NCORES = 8
NSH = N_NODES // NCORES          # nodes per core
WALKS = WALKS_PER_VERTEX * NSH   # walks per core (125000)
P = 128
COLS = (WALKS + P - 1) // P      # 977
PAD = P * COLS                   # 125056
NCH = 8
CHB = [(c * COLS) // NCH for c in range(NCH + 1)]   # chunk col bounds

_cache = {}


def _build(n_steps):
    f32 = mybir.dt.float32
    i32 = mybir.dt.int32
    u8 = mybir.dt.uint8
    nc = bacc.Bacc("TRN2", debug=False)

    tab_d = nc.dram_tensor("tab", [N_NODES * DEGREE, 1], f32, kind="ExternalInput")
    off_d = nc.dram_tensor("off", [(n_steps + 2) * P, COLS], u8, kind="ExternalInput")
    v0_d = nc.dram_tensor("v0", [P, COLS], f32, kind="ExternalInput")
    w_d = nc.dram_tensor("walks", [n_steps * P, COLS], f32, kind="ExternalOutput")

    v_bufs = [nc.alloc_sbuf_tensor(f"vb{s}", [P, COLS], f32).ap() for s in (0, 1)]
    qi_bufs = [nc.alloc_sbuf_tensor(f"qb{s}", [P, COLS], i32).ap() for s in (0, 1)]
    qf_bufs = [nc.alloc_sbuf_tensor(f"qf{s}", [P, COLS], f32).ap() for s in (0, 1)]
    of_bufs = [nc.alloc_sbuf_tensor(f"ob{s}", [P, COLS], u8).ap() for s in (0, 1)]
    off_f32 = [nc.alloc_sbuf_tensor(f"of{s}", [P, COLS], f32).ap() for s in (0, 1)]

    g_sems = [nc.alloc_semaphore(name=f"gsem{c}") for c in range(NCH)]
    r_sems = [nc.alloc_semaphore(name=f"rsem{c}") for c in range(NCH)]
    d_sems = [nc.alloc_semaphore(name=f"dsem{c}") for c in range(NCH)]
    off_sem = nc.alloc_semaphore(name="offsem")

    # prologue: v0 per chunk (boosts g_sems by 16), off(0), off(1)
    for c in range(NCH):
        lo, hi = CHB[c], CHB[c + 1]
        nc.sync.dma_start(v_bufs[0][:, lo:hi], v0_d.ap()[:, lo:hi]).then_inc(g_sems[c], 16)
    nc.sync.dma_start(of_bufs[0][:], off_d.ap()[0:P, :]).then_inc(off_sem, 16)
    nc.sync.dma_start(of_bufs[1][:], off_d.ap()[ds(P, P), :]).then_inc(off_sem, 16)

    ALL = [mybir.EngineType.Pool, mybir.EngineType.DVE, mybir.EngineType.SP]

    def step_body(i, s):
        # t = 2*i + s
        cur = v_bufs[s]
        nxt = v_bufs[1 - s]
        qi = qi_bufs[s]
        qf = qf_bufs[s]
        ofb = of_bufs[s]
        off32 = off_f32[s]
        t_1 = i * 2 + s                      # t
        row0 = i * (2 * P) + s * P           # t*128

        # --- DVE: per chunk, q = 16*v + off (f32 exact), cast i32 ---
        nc.vector.wait_ge(off_sem, i * 16 * 2 + s * 16 + 16)   # off_t present
        for c in range(NCH):
            lo, hi = CHB[c], CHB[c + 1]
            ncol = hi - lo
            nc.vector.wait_ge(g_sems[c], i * (2 * 16 * ncol) + s * 16 * ncol + 16)
            nc.vector.tensor_copy(off32[:, lo:hi], ofb[:, lo:hi])          # u8 -> f32
            nc.vector.tensor_scalar_mul(qf[:, lo:hi], cur[:, lo:hi], 16.0)
            nc.vector.tensor_tensor(qf[:, lo:hi], qf[:, lo:hi], off32[:, lo:hi],
                                    op=mybir.AluOpType.add)
            nc.vector.tensor_copy(qi[:, lo:hi], qf[:, lo:hi]).then_inc(d_sems[c], 1)

        # --- Pool: per chunk, per-column indirect gathers ---
        for c in range(NCH):
            lo, hi = CHB[c], CHB[c + 1]
            nc.gpsimd.wait_ge(d_sems[c], t_1 + 1)
            nc.gpsimd.wait_ge(r_sems[c], i * 32 + s * 16)
            for k in range(lo, hi):
                nc.gpsimd.indirect_dma_start(
                    out=nxt[:, k:k + 1],
                    out_offset=None,
                    in_=tab_d.ap()[:, :],
                    in_offset=bass.IndirectOffsetOnAxis(ap=qi[:, k:k + 1], axis=0),
                ).then_inc(g_sems[c], 16)

        # --- sync: record v_t per chunk; prefetch off(t+2) ---
        for c in range(NCH):
            lo, hi = CHB[c], CHB[c + 1]
            ncol = hi - lo
            nc.sync.wait_ge(g_sems[c], i * (2 * 16 * ncol) + s * 16 * ncol + 16)
            nc.sync.dma_start(w_d.ap()[ds(row0, P), lo:hi],
                              cur[:, lo:hi]).then_inc(r_sems[c], 16)
        nc.sync.wait_ge(d_sems[NCH - 1], t_1 + 1)
        nc.sync.dma_start(ofb[:], off_d.ap()[ds(row0 + 2 * P, P), :]).then_inc(off_sem, 16)

    assert n_steps % 2 == 0
    with nc.Fori(0, n_steps // 2, engines=ALL) as i:
        step_body(i, 0)
        step_body(i, 1)

    for c in range(NCH):
        ncol = CHB[c + 1] - CHB[c]
        nc.sync.wait_ge(g_sems[c], 16 * ncol * n_steps + 16)
        nc.sync.wait_ge(r_sems[c], 16 * n_steps)
    nc.all_engine_barrier()
    nc.finalize()
    return nc


def _get_nc(n_steps):
    if n_steps not in _cache:
        _cache[n_steps] = _build(n_steps)
    return _cache[n_steps]


def _build_timing(trip_steps):
    """Same per-step structure, but records/off-loads wrap onto fixed rows so
    I/O size is constant regardless of trip count. For step-scaling timing."""
    f32 = mybir.dt.float32
    i32 = mybir.dt.int32
    u8 = mybir.dt.uint8
    nc = bacc.Bacc("TRN2", debug=False)

    tab_d = nc.dram_tensor("tab", [N_NODES * DEGREE, 1], f32, kind="ExternalInput")
    off_d = nc.dram_tensor("off", [4 * P, COLS], u8, kind="ExternalInput")
    v0_d = nc.dram_tensor("v0", [P, COLS], f32, kind="ExternalInput")
    w_d = nc.dram_tensor("walks", [2 * P, COLS], f32, kind="ExternalOutput")

    v_bufs = [nc.alloc_sbuf_tensor(f"vb{s}", [P, COLS], f32).ap() for s in (0, 1)]
    qi_bufs = [nc.alloc_sbuf_tensor(f"qb{s}", [P, COLS], i32).ap() for s in (0, 1)]
    qf_bufs = [nc.alloc_sbuf_tensor(f"qf{s}", [P, COLS], f32).ap() for s in (0, 1)]
    of_bufs = [nc.alloc_sbuf_tensor(f"ob{s}", [P, COLS], u8).ap() for s in (0, 1)]
    off_f32 = [nc.alloc_sbuf_tensor(f"of{s}", [P, COLS], f32).ap() for s in (0, 1)]

    g_sems = [nc.alloc_semaphore(name=f"gsem{c}") for c in range(NCH)]
    r_sems = [nc.alloc_semaphore(name=f"rsem{c}") for c in range(NCH)]
    d_sems = [nc.alloc_semaphore(name=f"dsem{c}") for c in range(NCH)]
    off_sem = nc.alloc_semaphore(name="offsem")

    for c in range(NCH):
        lo, hi = CHB[c], CHB[c + 1]
        nc.sync.dma_start(v_bufs[0][:, lo:hi], v0_d.ap()[:, lo:hi]).then_inc(g_sems[c], 16)
    nc.sync.dma_start(of_bufs[0][:], off_d.ap()[0:P, :]).then_inc(off_sem, 16)
    nc.sync.dma_start(of_bufs[1][:], off_d.ap()[ds(P, P), :]).then_inc(off_sem, 16)

    ALL = [mybir.EngineType.Pool, mybir.EngineType.DVE, mybir.EngineType.SP]

    def step_body(i, s):
        cur = v_bufs[s]
        nxt = v_bufs[1 - s]
        qi = qi_bufs[s]
        qf = qf_bufs[s]
        ofb = of_bufs[s]
        off32 = off_f32[s]
        t_1 = i * 2 + s
        row0 = s * P                        # wrap: fixed rows per parity

        nc.vector.wait_ge(off_sem, i * 32 + s * 16 + 16)
        for c in range(NCH):
            lo, hi = CHB[c], CHB[c + 1]
            ncol = hi - lo
            nc.vector.wait_ge(g_sems[c], i * (2 * 16 * ncol) + s * 16 * ncol + 16)
            nc.vector.tensor_copy(off32[:, lo:hi], ofb[:, lo:hi])
            nc.vector.tensor_scalar_mul(qf[:, lo:hi], cur[:, lo:hi], 16.0)
            nc.vector.tensor_tensor(qf[:, lo:hi], qf[:, lo:hi], off32[:, lo:hi],
                                    op=mybir.AluOpType.add)
            nc.vector.tensor_copy(qi[:, lo:hi], qf[:, lo:hi]).then_inc(d_sems[c], 1)

        for c in range(NCH):
            lo, hi = CHB[c], CHB[c + 1]
            nc.gpsimd.wait_ge(d_sems[c], t_1 + 1)
            nc.gpsimd.wait_ge(r_sems[c], i * 32 + s * 16)
            for k in range(lo, hi):
                nc.gpsimd.indirect_dma_start(
                    out=nxt[:, k:k + 1],
                    out_offset=None,
                    in_=tab_d.ap()[:, :],
                    in_offset=bass.IndirectOffsetOnAxis(ap=qi[:, k:k + 1], axis=0),
                ).then_inc(g_sems[c], 16)

        for c in range(NCH):
            lo, hi = CHB[c], CHB[c + 1]
            ncol = hi - lo
            nc.sync.wait_ge(g_sems[c], i * (2 * 16 * ncol) + s * 16 * ncol + 16)
            nc.sync.dma_start(w_d.ap()[ds(row0, P), lo:hi],
                              cur[:, lo:hi]).then_inc(r_sems[c], 16)
        nc.sync.wait_ge(d_sems[NCH - 1], t_1 + 1)
        nc.sync.dma_start(ofb[:], off_d.ap()[ds((2 + s) * P, P), :]).then_inc(off_sem, 16)

    assert trip_steps % 2 == 0
    with nc.Fori(0, trip_steps // 2, engines=ALL) as i:
        step_body(i, 0)
        step_body(i, 1)

    for c in range(NCH):
        ncol = CHB[c + 1] - CHB[c]
        nc.sync.wait_ge(g_sems[c], 16 * ncol * trip_steps + 16)
        nc.sync.wait_ge(r_sems[c], 16 * trip_steps)
    nc.all_engine_barrier()
    nc.finalize()
    return nc


def timing_inputs():
    rng = np.random.default_rng(0)
    tab = rng.integers(0, N_NODES, size=(N_NODES * DEGREE, 1)).astype(np.float32)
    off = rng.integers(0, DEGREE, size=(4 * P, COLS)).astype(np.uint8)
    v0 = rng.integers(0, N_NODES, size=(P, COLS)).astype(np.float32)
    return [{"tab": tab, "off": off, "v0": v0} for _ in range(NCORES)]


def _prep_inputs(indptr, indices, rand_vals, n_steps=None):
    """Host prep: off = floor(16*u) as uint8 in [steps, P, COLS] layout."""
    rand_vals = np.asarray(rand_vals)
    W, N, L = rand_vals.shape
    if n_steps is None:
        n_steps = L
    tab = np.ascontiguousarray(np.asarray(indices).astype(np.float32).reshape(-1, 1))

    off_all = (rand_vals * DEGREE).astype(np.uint8)  # [W, N, L], values 0..15

    in_maps = []
    for c in range(NCORES):
        sl = off_all[:, c * NSH:(c + 1) * NSH, :]              # [W, NSH, L]
        U = sl.reshape(WALKS, L)[:, :n_steps]                  # walk-major
        U_pad = np.zeros((PAD, n_steps), np.uint8)
        U_pad[:WALKS] = U
        o_pre = U_pad.T.reshape(n_steps, COLS, P).transpose(0, 2, 1)  # [L,P,COLS]
        o_full = np.zeros(((n_steps + 2) * P, COLS), np.uint8)
        o_full[:n_steps * P] = o_pre.reshape(n_steps * P, COLS)

        j = np.arange(PAD)
        v0 = np.where(j < WALKS, c * NSH + (j % NSH), 0).astype(np.float32)
        v0 = v0.reshape(COLS, P).T.copy()

        in_maps.append({"tab": tab, "off": np.ascontiguousarray(o_full), "v0": v0})
    return in_maps


def kernel(indptr, indices, rand_vals):
    indptr = np.asarray(indptr)
    rand_vals = np.asarray(rand_vals)
    W, N, L = rand_vals.shape
    assert (W, N) == (WALKS_PER_VERTEX, N_NODES) and L % 2 == 0
    assert np.array_equal(indptr, (np.arange(N + 1) * DEGREE).astype(np.int32))

    in_maps = _prep_inputs(indptr, indices, rand_vals)
    nc = _get_nc(L)
    res = bass_utils.run_bass_kernel_spmd(nc, in_maps, core_ids=list(range(NCORES)))

    out = np.empty((W, N, L), np.float32)
    for c in range(NCORES):
        w_t = res.results[c]["walks"]                          # [L*P, COLS]
        Wc = w_t.reshape(L, P, COLS).transpose(0, 2, 1).reshape(L, PAD)[:, :WALKS]
        out[:, c * NSH:(c + 1) * NSH, :] = Wc.T.reshape(W, NSH, L)
    return out
